# revision 1
# baseline (speedup 1.0000x reference)
"""Back-projection (nn_BackProjectionNet) Trainium2 Bass kernel.

Full inputs in, full outputs out. Sharding: z (last dim, 192) split over 8
cores, 24 z-planes each; no collectives (each core fully independent).

Math: out[y,x,z] = sum_n lerp_x(padded_slice_n; sx_n(y,x)) / (120 + 1e-11)
with sx = sp*yc + cp*xc + C (the broadcast-along-y collapses the reference's
4-tap bilinear to an exact 2-tap x-lerp; the cropped region is always
in-bounds so the validity masks and the norm are trivial: norm == 120).

The 120 angles fold 8-ways (quarter-turn / mirror / pi-shift symmetries of
the uniform angle set) onto 16 base angles in [0,45deg] ("sin convention":
x-coeff cpa = sin a <= .707 so the tap window fits 128 partitions). Per base
angle and mirror-variant (xm), with S[y] = floor(spa*yc+C)+KMIN and
K[x] = floor(cpx*xc)-KMIN, p = frac_y + frac_x in [0,2):

  out += G0 + min(p,1)*(G1-G0) + max(p-1,0)*(G2-G1),  Gj = C4[S[y]+K[x]+j]

Device pipeline per (base, y-quarter):
  1. shear-DMA (per S-run, stride-0 y-replication) builds
     T[u=128, (d96, y32)] bf16 from DRAM combined-slice rows
  2. PE: one-hot matrices E0 / ED1 / ED2 (bf16, exact) expand taps:
     E0 accumulates sum-over-angles of G0 straight into PSUM;
     ED1/ED2 produce (G1-G0), (G2-G1) into a transient PSUM
  3. DVE/GPSIMD: M = Wplane (.) PD ; acc_w += M   (Wplanes precomputed)
  4. evac: OUT-quarter = acc_psum + acc_w_dve + acc_w_pool
Unwind: frame B is PE-transposed and added, scale by 1/(120+1e-11).
Host does only layout (pad, z-shard, slice flips, final transpose).
"""

import math
import numpy as np

import concourse.bacc as bacc
import concourse.mybir as mybir
from concourse import tile
from concourse.ap import AP
from concourse.bass_utils import run_bass_kernel_spmd

NA, LR, LZ, PAD = 120, 128, 192, 27
LP = LR + 2 * PAD          # 182
CEN = (LP - 1) / 2.0       # 90.5
N_CORES = 8
ZC = LZ // N_CORES         # 24
CROWS = 288                # padded combined-slice rows (max read ~282)
NBASE = 16
NSLOT = 4                  # job slots per base in C buffer
DT = mybir.dt
INV_NORM = float(np.float32(1.0 / (120.0 + 1e-11)))

F32, BF16 = DT.float32, DT.bfloat16
CJ = NBASE * NSLOT * ZC    # C row stride in elements = 1536


# ---------------------------------------------------------------- host math

def _job_slots():
    """Per base: slot -> (plain_member, flipped_member) or None.
    Slot order: [plain-A, plain-B, xm-A, xm-B]."""
    slots = {}
    for b in range(NBASE):
        if b == 0:
            slots[b] = [(60, 0), (90, 30), None, None]
        elif b == 15:
            slots[b] = [(75, 15), None, None, (105, 45)]
        else:
            slots[b] = [((60 + b) % 120, b), (90 - b, 30 - b),
                        (60 - b, 120 - b), (90 + b, 30 + b)]
    return slots


def _base_tables(b, xm):
    a = 2 * math.pi * b / NA
    cpa, spa = math.sin(a), math.cos(a)
    cpx = -cpa if xm else cpa
    yc = np.arange(PAD, PAD + LR, dtype=np.float64) - CEN
    xc = np.arange(PAD, PAD + LR, dtype=np.float64) - CEN
    ay = spa * yc + CEN
    bx = cpx * xc
    Sf, Kf = np.floor(ay), np.floor(bx)
    KMIN = int(Kf.min())
    K = (Kf - KMIN).astype(np.int64)        # [x] >= 0
    S = (Sf + KMIN).astype(np.int64)        # [y]
    r = ay - Sf
    k = bx - Kf
    p = (r[:, None] + k[None, :])           # [y,x] in [0,2)
    wA = np.minimum(p, 1.0).astype(np.float32)        # weight on (G1-G0)
    wB = np.maximum(p - 1.0, 0.0).astype(np.float32)  # weight on (G2-G1)
    return S, K, wA, wB


def host_prep():
    """Build all constant tables + plans."""
    slots = _job_slots()
    sets = []            # (b, xm)
    for b in range(NBASE):
        sets.append((b, False))
        if b != 0:
            sets.append((b, True))
    nset = len(sets)     # 31
    E = np.zeros((nset, 3, 128, 128), np.float32)
    W = np.zeros((nset, 2, 128, 128), np.float32)
    Sbase = {}
    for i, (b, xm) in enumerate(sets):
        S, K, wA, wB = _base_tables(b, xm)
        Sbase[b] = S
        for j in range(3):
            oh = np.zeros((128, 128), np.float32)
            oh[K + j, np.arange(128)] = 1.0    # K+j <= 91 < 128
            E[i, j] = oh
        E[i, 2] -= E[i, 1]
        E[i, 1] -= E[i, 0]
        W[i, 0] = wA.T            # [x, y]
        W[i, 1] = wB.T
    ident = np.eye(128, dtype=np.float32)
    runs = {}
    for b in range(NBASE):
        S = Sbase[b]
        for q in range(4):
            rr = []
            y = 32 * q
            while y < 32 * (q + 1):
                y1 = y
                while y1 + 1 < 32 * (q + 1) and S[y1 + 1] == S[y]:
                    y1 += 1
                rr.append((y, y1 - y + 1, int(S[y])))
                y = y1 + 1
            runs[(b, q)] = rr
    skip2 = {i: bool(np.all(W[i, 1] == 0)) for i in range(nset)}
    return dict(slots=slots, sets=sets, E=E, W=W, ident=ident,
                runs=runs, skip2=skip2)


def host_inputs(image, core):
    """Per-core input arrays. image [1,120,128,192] f32."""
    z0 = core * ZC
    img = np.asarray(image)[0, :, :, z0:z0 + ZC]               # [120,128,ZC]
    img_p = np.pad(img, ((0, 0), (PAD, PAD), (0, 0)))          # [120,182,ZC]
    slots = _job_slots()
    sp = np.zeros((NBASE * NSLOT, LP, ZC), np.float32)
    sf = np.zeros((NBASE * NSLOT, LP, ZC), np.float32)
    for b in range(NBASE):
        for s in range(NSLOT):
            j = slots[b][s]
            if j is None:
                continue
            mp, mf = j
            sp[b * NSLOT + s] = img_p[mp]
            sf[b * NSLOT + s] = img_p[mf][::-1]
    return {"slices_p": sp, "slices_f": sf}


# ---------------------------------------------------------------- device

def build_nc(tabs, repeat=1, nbases=NBASE, nquarters=4):
    sets, runs, skip2 = tabs["sets"], tabs["runs"], tabs["skip2"]
    nset = len(sets)
    set_idx = {bs: i for i, bs in enumerate(sets)}

    nc = bacc.Bacc("TRN2", target_bir_lowering=False, debug=False,
                   num_devices=N_CORES)
    d_sp = nc.dram_tensor("slices_p", [NBASE * NSLOT, LP, ZC], F32,
                          kind="ExternalInput")
    d_sf = nc.dram_tensor("slices_f", [NBASE * NSLOT, LP, ZC], F32,
                          kind="ExternalInput")
    d_E = nc.dram_tensor("e_tab", [nset * 3, 128, 128], BF16,
                         kind="ExternalInput")
    d_W = nc.dram_tensor("w_tab", [nset * 2, 128, 128], F32,
                         kind="ExternalInput")
    d_I = nc.dram_tensor("ident", [128, 128], F32, kind="ExternalInput")
    d_out = nc.dram_tensor("out", [128, 128, ZC], F32, kind="ExternalOutput")

    with tile.TileContext(nc) as tc:
        with tc.tile_pool(name="const", bufs=1) as cpool, \
             tc.tile_pool(name="work", bufs=3) as wpool, \
             tc.tile_pool(name="once", bufs=1) as opool, \
             tc.tile_pool(name="accs", bufs=1) as apool, \
             tc.tile_pool(name="dram", bufs=1, space="DRAM") as dpool, \
             tc.tile_pool(name="psum", bufs=1, space="PSUM") as ppool:

            d_C = dpool.tile([CROWS * NBASE * NSLOT * ZC], BF16, tag="cbuf")
            c_base = d_C[:].tensor

            # ---- constants to SBUF (outside timing loop) ----
            t_E = cpool.tile([128, nset * 3 * 128], BF16, tag="etab")
            nc.sync.dma_start(
                out=t_E[:],
                in_=AP(d_E[:].tensor, 0,
                       [[128, 128], [128 * 128, nset * 3], [1, 128]]))
            t_W = cpool.tile([128, nset * 2 * 128], F32, tag="wtab")
            nc.sync.dma_start(
                out=t_W[:],
                in_=AP(d_W[:].tensor, 0,
                       [[128, 128], [128 * 128, nset * 2], [1, 128]]))
            t_I = cpool.tile([128, 128], F32, tag="ident")
            nc.sync.dma_start(out=t_I[:], in_=d_I[:])

            def E_ap(si, j):      # lhsT [128, 128] bf16
                return t_E[:, (si * 3 + j) * 128:(si * 3 + j + 1) * 128]

            def W_ap(si, pl, q):  # [128, (d48 bcast), (y32)] f32
                base = (si * 2 + pl) * 128 + 32 * q
                return AP(t_W[:].tensor, base,
                          [[nset * 2 * 128, 128], [0, 48], [1, 32]])

            def body():
                # ---- zero C buffer ----
                t_z = opool.tile([128, 3456], BF16, tag="zero")
                nc.vector.memset(t_z[:], 0)
                nc.sync.dma_start(
                    out=AP(c_base, 0, [[3456, 128], [1, 3456]]),
                    in_=t_z[:])
                # ---- combine slices: C = P + flip(F) (host pre-flipped) ----
                t_p = opool.tile([64, LP * ZC], F32, tag="slp")
                t_f = opool.tile([64, LP * ZC], F32, tag="slf")
                nc.sync.dma_start(
                    out=t_p[:],
                    in_=AP(d_sp[:].tensor, 0, [[LP * ZC, 64], [1, LP * ZC]]))
                nc.sync.dma_start(
                    out=t_f[:],
                    in_=AP(d_sf[:].tensor, 0, [[LP * ZC, 64], [1, LP * ZC]]))
                t_c = opool.tile([64, LP * ZC], BF16, tag="slc")
                nc.vector.tensor_add(t_c[:], t_p[:], t_f[:])
                nc.sync.dma_start(
                    out=AP(c_base, 0, [[ZC, 64], [CJ, LP], [1, ZC]]),
                    in_=t_c[:])

                # ---- main loop ----
                out_t = apool.tile([128, 128 * 48], F32, tag="outbuf")
                for q in range(nquarters):
                    acc = ppool.tile([128, 1536], F32, tag="acc")
                    aw_d = apool.tile([128, 1536], F32, tag="aw_d")
                    aw_p = apool.tile([128, 1536], F32, tag="aw_p")
                    nc.gpsimd.memset(aw_d[:], 0)
                    nc.gpsimd.memset(aw_p[:], 0)
                    first_acc = [True] * 3
                    n_accmm = sum(3 * (1 if b == 0 else 2)
                                  for b in range(nbases))
                    mm_done = [0] * 3
                    for b in range(nbases):
                        # shear-DMA -> T [128u, (d96, y32)] bf16
                        t_T = wpool.tile([128, 96 * 32], BF16, tag="tshear")
                        tt = t_T[:].tensor
                        for (y0, ylen, S) in runs[(b, q)]:
                            nc.sync.dma_start(
                                out=AP(tt, (y0 - 32 * q) * 96,
                                       [[96 * 32, 128], [96, ylen], [1, 96]]),
                                in_=AP(c_base, S * CJ + b * NSLOT * ZC,
                                       [[CJ, 128], [0, ylen], [1, 96]]))
                        b_sets = [(b, False)] + ([(b, True)] if b != 0 else [])
                        for (bb, xm) in b_sets:
                            si = set_idx[(bb, xm)]
                            doff = 48 if xm else 0

                            def rhs_ap(ch):
                                return AP(tt, doff + ch * 16,
                                          [[96 * 32, 128], [1, 16], [96, 32]])

                            # acc stream: E0 accumulates G0 over all angles
                            for ch in range(3):
                                mm_done[ch] += 1
                                nc.tensor.matmul(
                                    acc[:, 512 * ch:512 * (ch + 1)],
                                    E_ap(si, 0), rhs_ap(ch),
                                    start=first_acc[ch],
                                    stop=(mm_done[ch] == n_accmm))
                                first_acc[ch] = False
                            # PD streams
                            for pl in (0, 1):
                                if pl == 1 and skip2[si]:
                                    continue
                                pd = ppool.tile([128, 1536], F32, tag="pd")
                                for ch in range(3):
                                    nc.tensor.matmul(
                                        pd[:, 512 * ch:512 * (ch + 1)],
                                        E_ap(si, pl + 1), rhs_ap(ch),
                                        start=True, stop=True)
                                m = wpool.tile([128, 1536], F32, tag="mbuf")
                                m3 = AP(m[:].tensor, 0,
                                        [[1536, 128], [32, 48], [1, 32]])
                                pd3 = AP(pd[:].tensor, 0,
                                         [[1536, 128], [32, 48], [1, 32]])
                                nc.vector.tensor_mul(m3, pd3, W_ap(si, pl, q))
                                if pl == 0:
                                    nc.gpsimd.tensor_add(aw_p[:], aw_p[:], m[:])
                                else:
                                    nc.gpsimd.tensor_add(aw_d[:], aw_d[:], m[:])
                    # evac quarter: OUT[(y32q),d48] = acc + aw_d + aw_p
                    nc.vector.tensor_add(aw_d[:], aw_d[:], aw_p[:])
                    nc.vector.tensor_add(
                        AP(out_t[:].tensor, 32 * q * 48,
                           [[128 * 48, 128], [1, 48], [48, 32]]),
                        AP(acc[:].tensor, acc[:].offset,
                           [[1536, 128], [32, 48], [1, 32]]),
                        AP(aw_d[:].tensor, aw_d[:].offset,
                           [[1536, 128], [32, 48], [1, 32]]))

                # ---- unwind: out = (A + B^T) * inv_norm ----
                for zc2 in range(2):
                    bt = ppool.tile([128, 1536], F32, tag="pd")
                    for zl in range(12):
                        z = zc2 * 12 + zl
                        nc.tensor.transpose(
                            bt[:, 128 * zl:128 * (zl + 1)],
                            AP(out_t[:].tensor, 24 + z,
                               [[128 * 48, 128], [48, 128]]),
                            t_I[:])
                    t_fin = opool.tile([128, 128 * 12], F32, tag="fin")
                    nc.vector.tensor_add(
                        t_fin[:],
                        AP(out_t[:].tensor, zc2 * 12,
                           [[128 * 48, 128], [48, 128], [1, 12]]),
                        AP(bt[:].tensor, bt[:].offset,
                           [[1536, 128], [1, 128], [128, 12]]))
                    nc.vector.tensor_scalar_mul(t_fin[:], t_fin[:], INV_NORM)
                    nc.sync.dma_start(
                        out=AP(d_out[:].tensor, zc2 * 12,
                               [[128 * 24, 128], [24, 128], [1, 12]]),
                        in_=t_fin[:])

            if repeat == 1:
                body()
            else:
                with tc.For_i(0, repeat, 1):
                    body()

    nc.compile()
    return nc


# ---------------------------------------------------------------- entry

_CACHE = {}


def _get(repeat=1):
    key = ("k", repeat)
    if key not in _CACHE:
        tabs = host_prep()
        nc = build_nc(tabs, repeat=repeat)
        _CACHE[key] = (tabs, nc)
    return _CACHE[key]


def make_in_maps(tabs, image):
    import ml_dtypes
    e_bf16 = np.ascontiguousarray(
        tabs["E"].reshape(-1, 128, 128)).astype(ml_dtypes.bfloat16)
    w_f32 = np.ascontiguousarray(tabs["W"].reshape(-1, 128, 128))
    in_maps = []
    for c in range(N_CORES):
        m = host_inputs(image, c)
        m["e_tab"] = e_bf16
        m["w_tab"] = w_f32
        m["ident"] = tabs["ident"]
        in_maps.append(m)
    return in_maps


def run_built(tabs, nc, image):
    in_maps = make_in_maps(tabs, image)
    res = run_bass_kernel_spmd(nc, in_maps, list(range(N_CORES)), trace=False)
    outs = []
    for c in range(N_CORES):
        o = res.results[c]["out"]                 # [x, y, ZC]
        outs.append(np.transpose(o, (1, 0, 2)))   # [y, x, ZC]
    full = np.concatenate(outs, axis=2)           # [128, 128, 192]
    return full[None].astype(np.float32)


def kernel(image):
    image = np.asarray(image, dtype=np.float32)
    tabs, nc = _get(repeat=1)
    return run_built(tabs, nc, image)

